# revision 12
# baseline (speedup 1.0000x reference)
"""Trainium2 Bass kernel: Swin-style window attention with relative position bias.

Self-contained: hardcodes B=64, N=576, C=768, H=12. Shards batch over 8 cores.

Per-core design (fully on-device compute; host only reshapes/sharding):
 - q/k projection in fp8e4 with DoubleRow perf mode (2 k-tiles of 128 per
   matmul at 0.5 cycles/row); weights pre-scaled by 16 (host) to dodge fp8
   subnormals; q/k sbuf tiles fp8 at 16x natural scale.
 - S_T[j,i] = k^T q per (batch, head) fp8 DoubleRow: k tile [d(64), 2, j]
   with slot1 zeroed, q broadcast stride-0 on the slot dim; the 16*16 and
   1/sqrt(d) factors fold into exp's scale = 1/2048.
 - softmax without max-subtract (logits bounded): P = exp(S_T/2048) * expb
   (exp written into the p2 tile, multiplied in place). The j-tail (j 512:576)
   of a head PAIR shares one psum tile (even head on partitions 0:64, odd on
   64:128) so its exp+mult are one instruction per pair, not two.
 - PV bf16: oD[65, i] = [v | 1]^T P_T accumulated over j-chunks; row 64 is
   the softmax denominator. v stored parity-major; the v-tail of a pair is
   packed [128, pair, 65] to match the shared tail layout.
 - Per-half division: gather 6 denom rows via one DMA, reciprocal, dram
   round-trip broadcast to 64 partitions (one stride-0 DMA), multiply in
   place, assemble oT via 2 partition-shift DMAs; project with bias.
 - All dram tensors host-packed to sbuf tile layout => one DMA per load.
 - Emission is software-pipelined per head pair: PV-phase of pair n runs
   interleaved with QK-phase of pair n+1; projection / next-batch qkv
   fills are scheduled into known PE stall points.
"""
import sys

sys.path.insert(0, "/opt/trn_rl_repo")

import numpy as np
import ml_dtypes

BF16 = ml_dtypes.bfloat16
F8 = ml_dtypes.float8_e4m3

B, N, C = 64, 576, 768
H, D = 12, 64
NCORES = 8
BL = B // NCORES           # 8 batches per core
NTOK = BL * N              # 4608 tokens per core
WS = 16.0                  # fp8 weight pre-scale (host)
EXPSCALE = 1.0 / (WS * WS * (D ** 0.5))  # = 1/2048

# token/j tiles of N=576: 4x128 + 1x64
JT = [(0, 128), (128, 128), (256, 128), (384, 128), (512, 64)]

_cache = {}


def _vhp(h):
    """Head index inside vpad/vps free dim (parity-major)."""
    return (h % 2) * 6 + h // 2


def _build(reps=1):
    key = ("nc", reps)
    if key in _cache:
        return _cache[key]
    from contextlib import ExitStack
    import concourse.tile as tile
    from concourse import bacc, mybir

    f32 = mybir.dt.float32
    bf16 = mybir.dt.bfloat16
    fp8 = mybir.dt.float8e4
    DR = mybir.MatmulPerfMode.DoubleRow
    EXP = mybir.ActivationFunctionType.Exp

    nc = bacc.Bacc("TRN2", target_bir_lowering=False, debug=False,
                   num_devices=NCORES)
    xT = nc.dram_tensor("xT", [128, 6, NTOK], bf16, kind="ExternalInput").ap()
    xT8 = nc.dram_tensor("xT8", [128, 3, 2, NTOK], fp8,
                         kind="ExternalInput").ap()
    wqk = nc.dram_tensor("wqk", [128, 3, 2, 2 * C], fp8,
                         kind="ExternalInput").ap()
    qkb = nc.dram_tensor("qkb", [128, 12], f32, kind="ExternalInput").ap()
    wv = nc.dram_tensor("wv", [128, 6, C], bf16, kind="ExternalInput").ap()
    wp = nc.dram_tensor("wp", [128, 6, C], bf16, kind="ExternalInput").ap()
    pb2 = nc.dram_tensor("pb2", [128, 6], f32, kind="ExternalInput").ap()
    expb = nc.dram_tensor("expb", [128, H, 4, N], bf16,
                          kind="ExternalInput").ap()
    expb4 = nc.dram_tensor("expb4", [128, 6, N], bf16,
                           kind="ExternalInput").ap()
    out = nc.dram_tensor("out", [128, 6, NTOK], bf16,
                         kind="ExternalOutput").ap()

    with tile.TileContext(nc) as tc, ExitStack() as ctx:
        ent = ctx.enter_context
        const = ent(tc.tile_pool(name="const", bufs=1))
        sps = ent(tc.tile_pool(name="sps", bufs=3, space="PSUM"))
        bigp = ent(tc.tile_pool(name="bigp", bufs=1, space="PSUM"))
        xbp = ent(tc.tile_pool(name="xb", bufs=2))
        xb8p = ent(tc.tile_pool(name="xb8", bufs=2))
        qtp = ent(tc.tile_pool(name="qt", bufs=10))
        ktp = ent(tc.tile_pool(name="kt", bufs=10))
        vpp = ent(tc.tile_pool(name="vpad", bufs=2))
        vp4 = ent(tc.tile_pool(name="vpad4", bufs=2))
        p2p = ent(tc.tile_pool(name="p2p", bufs=12))
        odsp = ent(tc.tile_pool(name="odsall", bufs=2))
        otp = ent(tc.tile_pool(name="ot", bufs=2))
        yp = ent(tc.tile_pool(name="y", bufs=2))
        small1 = ent(tc.tile_pool(name="small1", bufs=1))
        rbp = ent(tc.tile_pool(name="rb", bufs=1))
        dramp = ent(tc.tile_pool(name="dram", bufs=2, space="DRAM"))

        # ---- constant tiles (single packed DMAs, emitted lazily below) ----
        wqk_sb = const.tile([128, 3, 2, 2 * C], fp8)
        wv_sb = const.tile([128, 6, C], bf16)
        wp_sb = const.tile([128, 6, C], bf16)
        qkb_sb = const.tile([128, 12], f32)
        pb2_sb = const.tile([128, 6], f32)
        expb_sb = const.tile([128, H, 4, N], bf16)
        expb4_sb = const.tile([128, 6, N], bf16)

        def load_wqk():
            nc.gpsimd.dma_start(wqk_sb[:, :, :, :], wqk[:, :, :, :])

        def load_wv_qkb():
            nc.gpsimd.dma_start(wv_sb[:, :, :], wv[:, :, :])
            nc.gpsimd.dma_start(qkb_sb[:, :], qkb[:, :])
            nc.gpsimd.dma_start(pb2_sb[:, :], pb2[:, :])

        def load_wp_expb():
            nc.gpsimd.dma_start(wp_sb[:, :, :], wp[:, :, :])
            nc.gpsimd.dma_start(expb_sb[:, :, :, :], expb[:, :, :, :])
            nc.gpsimd.dma_start(expb4_sb[:, :, :], expb4[:, :, :])

        # ---- per-batch state (keyed by flattened batch index) ----
        st = {}

        def emit_load_xb(i, b):
            xb = xbp.tile([128, 6, N], bf16, tag="xb")
            nc.sync.dma_start(xb[:, :, :], xT[:, :, b * N:(b + 1) * N])
            xb8 = xb8p.tile([128, 3, 2, N], fp8, tag="xb8")
            nc.gpsimd.dma_start(xb8[:, :, :, :],
                                xT8[:, :, :, b * N:(b + 1) * N])
            st[(i, "xb")] = xb
            st[(i, "xb8")] = xb8

        def emit_qkproj(i, mt):
            xb8 = st[(i, "xb8")]
            ps = sps.tile([128, 768], f32, name="qps", tag="s")
            for kc2 in range(3):
                for n0, nsz in [(0, 512), (512, 64)]:
                    nc.tensor.matmul(
                        ps[:, n0:n0 + nsz],
                        lhsT=wqk_sb[:, kc2, :, mt * 128:(mt + 1) * 128],
                        rhs=xb8[:, kc2, :, n0:n0 + nsz],
                        start=(kc2 == 0), stop=(kc2 == 2),
                        perf_mode=DR)
            if mt < 6:
                qt = qtp.tile([128, 1, N], fp8, tag="qt")
                nc.vector.tensor_scalar_add(qt[:, 0, :], ps[:, 0:N],
                                            qkb_sb[:, mt:mt + 1])
                st[(i, "qt", mt)] = qt
            else:
                kt = ktp.tile([128, 2, N], fp8, tag="kt")
                nc.gpsimd.memset(kt[:, 1, :], 0.0)
                nc.vector.tensor_copy(kt[:, 0, :], ps[:, 0:N])
                st[(i, "kt", mt - 6)] = kt

        def emit_vproj(i, tt, halfk):
            xb = st[(i, "xb")]
            if tt == 0 and halfk == 0:
                vpad = vpp.tile([128, 4, H, 65], bf16, tag="vpad")
                nc.vector.memset(vpad[:, :, :, 64:65], 1.0)
                st[(i, "vpad")] = vpad
                vpad4 = vp4.tile([128, 6, 65], bf16, tag="vpad4")
                nc.vector.memset(vpad4[:, :, 64:65], 1.0)
                st[(i, "vpad4")] = vpad4
            t0, tsz = JT[tt]
            if halfk == 0:
                st[(i, "vps", tt)] = sps.tile([128, 12, 64], f32, name="vps",
                                              tag="s")
            ps = st[(i, "vps", tt)]
            for kc in range(3 * halfk, 3 * halfk + 3):
                if tt < 4:
                    for g0, gn in [(0, 8), (8, 4)]:
                        nc.tensor.matmul(
                            ps[0:tsz, g0:g0 + gn, :],
                            lhsT=xb[:, kc, t0:t0 + tsz],
                            rhs=wv_sb[:, kc, g0 * 64:(g0 + gn) * 64],
                            start=(kc == 0), stop=(kc == 5))
                else:
                    # tail tokens: even heads -> psum partitions 0:64,
                    # odd heads -> 64:128 (paired layout for shared tail)
                    for par in range(2):
                        nc.tensor.matmul(
                            ps[64 * par:64 * par + 64, 0:6, :],
                            lhsT=xb[:, kc, t0:t0 + tsz],
                            rhs=wv_sb[:, kc, 384 * par:384 * par + 384],
                            start=(kc == 0), stop=(kc == 5),
                            skip_group_check=True)
            if halfk == 1:
                eng = nc.vector
                if tt < 4:
                    eng.tensor_copy(st[(i, "vpad")][0:tsz, tt, :, 0:64],
                                    ps[0:tsz, :, :])
                else:
                    eng.tensor_copy(st[(i, "vpad4")][:, :, 0:64],
                                    ps[:, 0:6, :])
                del st[(i, "vps", tt)]

        def emit_attn_qk(i, h, jt):
            base = (h % 2) * 64
            qv = st[(i, "qt", h // 2)]
            kv = st[(i, "kt", h // 2)]
            j0, jsz = JT[jt]
            s = sps.tile([128, 768], f32, name="satt", tag="s")
            for n0, nsz in [(0, 512), (512, 64)]:
                nc.tensor.matmul(
                    s[0:jsz, n0:n0 + nsz],
                    lhsT=kv[base:base + 64, :, j0:j0 + jsz],
                    rhs=qv[base:base + 64, :, n0:n0 + nsz].broadcast_to(
                        (64, 2, nsz)),
                    start=True, stop=True, perf_mode=DR)
            p2 = p2p.tile([128, N], bf16, tag="p2")
            nc.scalar.activation(p2[0:jsz, :], s[0:jsz, 0:N], EXP,
                                 scale=EXPSCALE)
            nc.vector.tensor_mul(p2[0:jsz, :], p2[0:jsz, :],
                                 expb_sb[0:jsz, h, jt, :])
            st[(i, "p2", h, jt)] = p2

        def emit_attn_qk_tail(i, hp):
            """jt=4 for both heads of pair hp: shared psum + one exp+mult."""
            he, ho = 2 * hp, 2 * hp + 1
            j0, jsz = JT[4]
            s = sps.tile([128, 768], f32, name="satt", tag="s")
            qv = st[(i, "qt", hp)]
            kv = st[(i, "kt", hp)]
            for n0, nsz in [(0, 512), (512, 64)]:
                # even head: DoubleRow into partitions 0:64
                nc.tensor.matmul(
                    s[0:jsz, n0:n0 + nsz],
                    lhsT=kv[0:64, :, j0:j0 + jsz],
                    rhs=qv[0:64, :, n0:n0 + nsz].broadcast_to((64, 2, nsz)),
                    start=True, stop=True, perf_mode=DR,
                    skip_group_check=True)
                # odd head: plain matmul into partitions 64:128 (DoubleRow
                # with dst partition base 64 fails the ISA check)
                nc.tensor.matmul(
                    s[64:64 + jsz, n0:n0 + nsz],
                    lhsT=kv[64:128, 0, j0:j0 + jsz],
                    rhs=qv[64:128, 0, n0:n0 + nsz],
                    start=True, stop=True,
                    skip_group_check=True)
            p2 = p2p.tile([128, N], bf16, tag="p2")
            nc.scalar.activation(p2[:, :], s[:, 0:N], EXP, scale=EXPSCALE)
            nc.vector.tensor_mul(p2[:, :], p2[:, :], expb4_sb[:, hp, :])
            st[(i, "p2", he, 4)] = p2
            st[(i, "p2", ho, 4)] = p2

        def emit_attn_pv(i, h, jt):
            j0, jsz = JT[jt]
            if jt == 0:
                st[(i, "od")] = bigp.tile([65, N], f32, name="od", tag="od")
            odp = st[(i, "od")]
            p2 = st.pop((i, "p2", h, jt))
            if jt < 4:
                lhsT = st[(i, "vpad")][0:jsz, jt, _vhp(h), :]
                rhs = [p2[0:jsz, n0:n0 + nsz] for n0, nsz in
                       [(0, 512), (512, 64)]]
            else:
                base = (h % 2) * 64
                lhsT = st[(i, "vpad4")][base:base + 64, h // 2, :]
                rhs = [p2[base:base + 64, n0:n0 + nsz] for n0, nsz in
                       [(0, 512), (512, 64)]]
            for (n0, nsz), r in zip([(0, 512), (512, 64)], rhs):
                nc.tensor.matmul(
                    odp[0:65, n0:n0 + nsz], lhsT=lhsT, rhs=r,
                    start=(jt == 0), stop=(jt == 4),
                    skip_group_check=True)
            if jt == 4:
                hh = h % 6
                if hh == 0:
                    st[(i, "odsall", h // 6)] = odsp.tile(
                        [65, 2, 3, N], bf16, name="odsall", tag="odsall")
                odsall = st[(i, "odsall", h // 6)]
                nc.vector.tensor_copy(odsall[:, hh % 2, hh // 2, :],
                                      odp[:, :])

        def emit_division(i, half):
            odsall = st[(i, "odsall", half)]
            if half == 0:
                ot = otp.tile([128, 6, N], bf16, tag="ot")
                st[(i, "ot")] = ot
            ot = st[(i, "ot")]
            d6 = small1.tile([6, N], bf16, tag="d6")
            nc.sync.dma_start(d6[:, :], odsall[64:65, :, :, :])
            rf32 = small1.tile([6, N], f32, tag="rf32")
            nc.vector.reciprocal(rf32[:, :], d6[:, :])
            rbf = small1.tile([6, N], bf16, tag="rbf")
            nc.vector.tensor_copy(rbf[:, :], rf32[:, :])
            rdram = dramp.tile([1, 6, N], bf16, tag="rdram")
            nc.sync.dma_start(rdram[0, :, :], rbf[:, :])
            rb6 = rbp.tile([64, 2, 3, N], bf16, tag="rb6")
            nc.sync.dma_start(rb6[:, :, :, :],
                              rdram[:, :, :].broadcast_to((64, 6, N)))
            for par in range(2):
                nc.vector.tensor_mul(odsall[0:64, par, :, :],
                                     odsall[0:64, par, :, :],
                                     rb6[:, par, :, :])
            nc.sync.dma_start(ot[0:64, 3 * half:3 * half + 3, :],
                              odsall[0:64, 0, :, :])
            nc.sync.dma_start(ot[64:128, 3 * half:3 * half + 3, :],
                              odsall[0:64, 1, :, :])

        def emit_proj(i, b, mt):
            ot = st[(i, "ot")]
            if mt == 0:
                st[(i, "y6")] = yp.tile([128, 6, N], bf16, name="y6",
                                        tag="y6")
            y6 = st[(i, "y6")]
            ps = sps.tile([128, 768], f32, name="pps", tag="s")
            for kc in range(6):
                for n0, nsz in [(0, 512), (512, 64)]:
                    nc.tensor.matmul(
                        ps[:, n0:n0 + nsz],
                        lhsT=wp_sb[:, kc, mt * 128:(mt + 1) * 128],
                        rhs=ot[:, kc, n0:n0 + nsz],
                        start=(kc == 0), stop=(kc == 5))
            nc.vector.tensor_scalar_add(y6[:, mt, :], ps[:, 0:N],
                                         pb2_sb[:, mt:mt + 1])
            if mt == 5:
                nc.sync.dma_start(out[:, :, b * N:(b + 1) * N], y6[:, :, :])

        # ---- head-pair software-pipelined emission ----
        from collections import deque
        batches = [(r * BL + b, b) for r in range(reps) for b in range(BL)]
        nbat = len(batches)
        pairs = [(i, b, hp) for (i, b) in batches for hp in range(6)]
        NP = len(pairs)

        # fill schedule: fills[(i, hp)] consumed during the slot whose
        # PV-phase is pair (i, hp)
        fills = {}
        for idx, (i, b) in enumerate(batches):
            items = []
            if idx + 1 < nbat:
                i2, b2 = batches[idx + 1]
                items.append(("s0", lambda i2=i2, b2=b2:
                              emit_load_xb(i2, b2)))
            if idx > 0:
                ip, bp = batches[idx - 1]
                for mt in range(6):
                    items.append(("x", lambda ip=ip, bp=bp, mt=mt:
                                  emit_proj(ip, bp, mt)))
            if idx + 1 < nbat:
                i2, b2 = batches[idx + 1]
                order = [("qk", 0), ("qk", 6), ("qk", 1), ("qk", 7),
                         ("v", 0), ("qk", 2), ("qk", 8), ("v", 1),
                         ("qk", 3), ("qk", 9), ("v", 2), ("qk", 4),
                         ("qk", 10), ("v", 3), ("qk", 5), ("qk", 11),
                         ("v", 4)]
                for kind, a in order:
                    if kind == "qk":
                        items.append(("x", lambda i2=i2, a=a:
                                      emit_qkproj(i2, a)))
                    else:
                        items.append(("x", lambda i2=i2, a=a:
                                      (emit_vproj(i2, a, 0),
                                       emit_vproj(i2, a, 1))))
            # distribute: slot 0 gets the xb load only; rest spread 1..5
            fl = {hp: [] for hp in range(6)}
            rest = []
            for kind, f in items:
                if kind == "s0":
                    fl[0].append(f)
                else:
                    rest.append(f)
            for n, f in enumerate(rest):
                fl[1 + (n * 5) // max(len(rest), 1)].append(f)
            for hp in range(6):
                fills[(i, hp)] = deque(fl[hp])

        load_wqk()
        load_wv_qkb()
        emit_load_xb(0, 0)
        for mt in range(12):
            emit_qkproj(0, mt)
        for tt in range(5):
            emit_vproj(0, tt, 0)
            emit_vproj(0, tt, 1)
        load_wp_expb()

        def qphase(s, k):
            """k-th QK-emission step (0..9) of the Q-side pair of slot s."""
            if s >= NP:
                return
            iq, bq, hq = pairs[s]
            he, ho = 2 * hq, 2 * hq + 1
            seq = [(he, 0), (ho, 0), (he, 1), (ho, 1), (he, 2),
                   (ho, 2), (he, 3), (ho, 3), (he, 4), (ho, 4)]
            h, jt = seq[k]
            if jt == 4:
                if h == ho:
                    emit_attn_qk_tail(iq, hq)
            else:
                emit_attn_qk(iq, h, jt)

        gfill = deque()

        for s in range(NP + 1):
            P = pairs[s - 1] if s >= 1 else None
            if P:
                gfill.extend(fills.get((P[0], P[2]), ()))
            f = gfill

            def pop():
                if f:
                    f.popleft()()

            pop()
            qphase(s, 0)
            if P:
                ip, bp, hp = P
                pe, po = 2 * hp, 2 * hp + 1
                emit_attn_pv(ip, pe, 0)
                qphase(s, 1)
                emit_attn_pv(ip, pe, 1)
                qphase(s, 2)
                emit_attn_pv(ip, pe, 2)
                pop()
                qphase(s, 3)
                emit_attn_pv(ip, pe, 3)
                qphase(s, 4)
                emit_attn_pv(ip, pe, 4)
                pop()
                qphase(s, 5)
                emit_attn_pv(ip, po, 0)
                qphase(s, 6)
                emit_attn_pv(ip, po, 1)
                qphase(s, 7)
                emit_attn_pv(ip, po, 2)
                pop()
                qphase(s, 8)
                emit_attn_pv(ip, po, 3)
                qphase(s, 9)
                emit_attn_pv(ip, po, 4)
                if hp == 5 and (s == NP or pairs[s][0] != ip):
                    while f:
                        f.popleft()()
                if hp == 2:
                    emit_division(ip, 0)
                elif hp == 5:
                    emit_division(ip, 1)
            else:
                for k in range(1, 10):
                    qphase(s, k)

        ilast, blast = batches[-1]
        for mt in range(6):
            emit_proj(ilast, blast, mt)

    nc.compile()
    _cache[key] = nc
    return nc


def _prep_inputs(x, qkv_w, q_bias, v_bias, rpb_table, rel_idx, proj_w, proj_b):
    x = np.asarray(x, np.float32)
    qkv_w = np.asarray(qkv_w, np.float32)
    q_bias = np.asarray(q_bias, np.float32)
    v_bias = np.asarray(v_bias, np.float32)
    rpb_table = np.asarray(rpb_table, np.float32)
    rel_idx = np.asarray(rel_idx)
    proj_w = np.asarray(proj_w, np.float32)
    proj_b = np.asarray(proj_b, np.float32)

    # q/k weights: natural magnitude (no attention scale), x WS for fp8,
    # packed [128, 3, 2, 1536]
    wqk_f = (qkv_w[:2 * C].T * WS).astype(F8)                       # [768,1536]
    wqk_np = np.ascontiguousarray(
        wqk_f.reshape(3, 2, 128, 2 * C).transpose(2, 0, 1, 3))
    qb = np.concatenate([q_bias * WS, np.zeros(C, np.float32)])     # [1536]
    qkb_np = np.ascontiguousarray(qb.reshape(12, 128).T)            # [128, 12]
    # wv columns parity-major: [even heads | odd heads]
    wv_f = qkv_w[2 * C:].T.astype(BF16)                             # [768, 768]
    wv_pm = np.ascontiguousarray(
        wv_f.reshape(C, 12, 64)[:, [0, 2, 4, 6, 8, 10, 1, 3, 5, 7, 9, 11],
                                :].reshape(C, C))
    wv_np = np.ascontiguousarray(
        wv_pm.reshape(6, 128, C).transpose(1, 0, 2))
    wp_np = np.ascontiguousarray(
        proj_w.T.astype(BF16).reshape(6, 128, C).transpose(1, 0, 2))
    pb_eff = (proj_b + proj_w @ v_bias).astype(np.float32)          # [768]
    pb2_np = np.ascontiguousarray(pb_eff.reshape(6, 128).T)         # [128, 6]
    et = np.exp(rpb_table)                     # [2209, 12]
    idx = np.clip(np.asarray(rel_idx, np.int64), 0, et.shape[0] - 1)
    g = et[idx]                                # [576i, 576j, 12]
    ghji = np.ascontiguousarray(g.transpose(2, 1, 0)).astype(BF16)  # [h, j, i]
    expb_np = np.zeros((128, H, 4, N), BF16)
    for jt, (j0, jsz) in enumerate(JT[:4]):
        expb_np[:, :, jt, :] = ghji[:, j0:j0 + jsz, :].transpose(1, 0, 2)
    # tail (j 512:576) packed per head pair: even head partitions 0:64
    expb4_np = np.zeros((128, 6, N), BF16)
    for hp in range(6):
        expb4_np[0:64, hp, :] = ghji[2 * hp, 512:576, :]
        expb4_np[64:128, hp, :] = ghji[2 * hp + 1, 512:576, :]

    in_maps = []
    for ci in range(NCORES):
        xc = x[ci * BL:(ci + 1) * BL]          # [8, 576, 768]
        xf = np.ascontiguousarray(
            xc.transpose(2, 0, 1).reshape(C, NTOK))
        xT_np = np.ascontiguousarray(
            xf.astype(BF16).reshape(6, 128, NTOK).transpose(1, 0, 2))
        xT8_np = np.ascontiguousarray(
            xf.astype(F8).reshape(3, 2, 128, NTOK).transpose(2, 0, 1, 3))
        in_maps.append({
            "xT": xT_np, "xT8": xT8_np,
            "wqk": wqk_np, "qkb": qkb_np,
            "wv": wv_np, "wp": wp_np, "pb2": pb2_np,
            "expb": expb_np, "expb4": expb4_np,
        })
    return in_maps


def kernel(x, qkv_w, q_bias, v_bias, rpb_table, rel_idx, proj_w, proj_b,
           _want_profile=False):
    in_maps = _prep_inputs(x, qkv_w, q_bias, v_bias, rpb_table, rel_idx,
                           proj_w, proj_b)
    nc = _build()
    from concourse.bass_utils import run_bass_kernel_spmd
    res = run_bass_kernel_spmd(nc, in_maps, core_ids=list(range(NCORES)),
                               trace=False)
    outs = [np.asarray(r["out"], np.float32).transpose(1, 0, 2).reshape(
        C, NTOK).T.reshape(BL, N, C) for r in res.results]
    y = np.concatenate(outs, 0)
    if _want_profile:
        return y, res
    return y


# revision 26
# speedup vs baseline: 4.1828x; 4.1828x over previous
"""Trainium2 Bass kernel: Swin-style window attention with relative position bias.

Self-contained: hardcodes B=64, N=576, C=768, H=12. Shards batch over 8 cores.

Per-core design (fully on-device compute; host only reshapes/sharding):
 - q/k projection in fp8e4 with DoubleRow perf mode (2 k-tiles of 128 per
   matmul at 0.5 cycles/row); weights pre-scaled by 16 (host) to dodge fp8
   subnormals; q/k sbuf tiles fp8 at 16x natural scale.
 - S_T[j,i] = k^T q per (batch, head) fp8 DoubleRow: k tile [d(64), 2, j]
   with slot1 zeroed, q broadcast stride-0 on the slot dim; the 16*16 and
   1/sqrt(d) factors fold into exp's scale = 1/2048.
 - softmax without max-subtract (logits bounded): P = exp(S_T/2048) * expb
   (exp written into the p2 tile, multiplied in place). The j-tail (j 512:576)
   of a head PAIR shares one psum tile (even head on partitions 0:64, odd on
   64:128) so its exp+mult are one instruction per pair, not two.
 - PV bf16: oD[65, i] = [v | 1]^T P_T accumulated over j-chunks; row 64 is
   the softmax denominator. v stored parity-major; the v-tail of a pair is
   packed [128, pair, 65] to match the shared tail layout.
 - Per-half division: gather 6 denom rows via one DMA, reciprocal, dram
   round-trip broadcast to 64 partitions (one stride-0 DMA), multiply in
   place, assemble oT via 2 partition-shift DMAs; project with bias.
 - All dram tensors host-packed to sbuf tile layout => one DMA per load.
 - Emission is software-pipelined per head pair: PV-phase of pair n runs
   interleaved with QK-phase of pair n+1; projection / next-batch qkv
   fills are scheduled into known PE stall points.
"""
import sys

sys.path.insert(0, "/opt/trn_rl_repo")

import numpy as np
import ml_dtypes

BF16 = ml_dtypes.bfloat16
F8 = ml_dtypes.float8_e4m3

B, N, C = 64, 576, 768
H, D = 12, 64
NCORES = 8
BL = B // NCORES           # 8 batches per core
NTOK = BL * N              # 4608 tokens per core
WS = 16.0                  # fp8 weight pre-scale (host)
EXPSCALE = 1.0 / (WS * WS * (D ** 0.5))  # = 1/2048

# token/j tiles of N=576: 4x128 + 1x64
JT = [(0, 128), (128, 128), (256, 128), (384, 128), (512, 64)]

_cache = {}


def _vhp(h):
    """Head index inside vpad/vps free dim (parity-major)."""
    return (h % 2) * 6 + h // 2


def _build(reps=1):
    key = ("nc", reps)
    if key in _cache:
        return _cache[key]
    from contextlib import ExitStack
    import concourse.tile as tile
    from concourse import bacc, mybir

    f32 = mybir.dt.float32
    bf16 = mybir.dt.bfloat16
    fp8 = mybir.dt.float8e4
    DR = mybir.MatmulPerfMode.DoubleRow
    EXP = mybir.ActivationFunctionType.Exp

    nc = bacc.Bacc("TRN2", target_bir_lowering=False, debug=False,
                   num_devices=NCORES)
    xT = nc.dram_tensor("xT", [128, 6, NTOK], bf16, kind="ExternalInput").ap()
    xT8 = nc.dram_tensor("xT8", [128, 3, 2, NTOK], fp8,
                         kind="ExternalInput").ap()
    wqk = nc.dram_tensor("wqk", [128, 3, 2, 2 * C], fp8,
                         kind="ExternalInput").ap()
    qkb = nc.dram_tensor("qkb", [128, 12], f32, kind="ExternalInput").ap()
    wv = nc.dram_tensor("wv", [128, 6, C], bf16, kind="ExternalInput").ap()
    wp = nc.dram_tensor("wp", [128, 6, C], bf16, kind="ExternalInput").ap()
    pb2 = nc.dram_tensor("pb2", [128, 6], f32, kind="ExternalInput").ap()
    expb = nc.dram_tensor("expb", [128, H, 4, N], bf16,
                          kind="ExternalInput").ap()
    expb4 = nc.dram_tensor("expb4", [128, 6, N], bf16,
                           kind="ExternalInput").ap()
    out = nc.dram_tensor("out", [128, 6, NTOK], bf16,
                         kind="ExternalOutput").ap()

    with tile.TileContext(nc) as tc, ExitStack() as ctx:
        ent = ctx.enter_context
        const = ent(tc.tile_pool(name="const", bufs=1))
        sps = ent(tc.tile_pool(name="sps", bufs=3, space="PSUM"))
        bigp = ent(tc.tile_pool(name="bigp", bufs=1, space="PSUM"))
        xbp = ent(tc.tile_pool(name="xb", bufs=2))
        xb8p = ent(tc.tile_pool(name="xb8", bufs=2))
        qtp = ent(tc.tile_pool(name="qt", bufs=10))
        ktp = ent(tc.tile_pool(name="kt", bufs=10))
        vpp = ent(tc.tile_pool(name="vpad", bufs=2))
        vp4 = ent(tc.tile_pool(name="vpad4", bufs=2))
        p2p = ent(tc.tile_pool(name="p2p", bufs=12))
        odsp = ent(tc.tile_pool(name="odsall", bufs=2))
        otp = ent(tc.tile_pool(name="ot", bufs=2))
        yp = ent(tc.tile_pool(name="y", bufs=2))
        small1 = ent(tc.tile_pool(name="small1", bufs=1))
        rbp = ent(tc.tile_pool(name="rb", bufs=1))
        dramp = ent(tc.tile_pool(name="dram", bufs=2, space="DRAM"))

        # ---- constant tiles (single packed DMAs, emitted lazily below) ----
        wqk_sb = const.tile([128, 3, 2, 2 * C], fp8)
        wv_sb = const.tile([128, 6, C], bf16)
        wp_sb = const.tile([128, 6, C], bf16)
        qkb_sb = const.tile([128, 12], f32)
        pb2_sb = const.tile([128, 6], f32)
        expb_sb = const.tile([128, H, 4, N], bf16)
        expb4_sb = const.tile([128, 6, N], bf16)

        def load_wqk():
            nc.gpsimd.dma_start(wqk_sb[:, :, :, :], wqk[:, :, :, :])

        def load_wv_qkb():
            nc.gpsimd.dma_start(wv_sb[:, :, :], wv[:, :, :])
            nc.gpsimd.dma_start(qkb_sb[:, :], qkb[:, :])
            nc.gpsimd.dma_start(pb2_sb[:, :], pb2[:, :])

        def load_wp_expb():
            nc.gpsimd.dma_start(wp_sb[:, :, :], wp[:, :, :])
            nc.gpsimd.dma_start(expb_sb[:, :, :, :], expb[:, :, :, :])
            nc.gpsimd.dma_start(expb4_sb[:, :, :], expb4[:, :, :])

        # ---- per-batch state (keyed by flattened batch index) ----
        st = {}

        def emit_load_xb(i, b):
            xb = xbp.tile([128, 6, N], bf16, tag="xb")
            nc.sync.dma_start(xb[:, :, :], xT[:, :, b * N:(b + 1) * N])
            xb8 = xb8p.tile([128, 3, 2, N], fp8, tag="xb8")
            nc.gpsimd.dma_start(xb8[:, :, :, :],
                                xT8[:, :, :, b * N:(b + 1) * N])
            st[(i, "xb")] = xb
            st[(i, "xb8")] = xb8

        def emit_qkproj(i, mt):
            xb8 = st[(i, "xb8")]
            ps = sps.tile([128, 768], f32, name="qps", tag="s")
            for kc2 in range(3):
                for n0, nsz in [(0, 512), (512, 64)]:
                    nc.tensor.matmul(
                        ps[:, n0:n0 + nsz],
                        lhsT=wqk_sb[:, kc2, :, mt * 128:(mt + 1) * 128],
                        rhs=xb8[:, kc2, :, n0:n0 + nsz],
                        start=(kc2 == 0), stop=(kc2 == 2),
                        perf_mode=DR)
            if mt < 6:
                qt = qtp.tile([128, 1, N], fp8, tag="qt")
                nc.vector.tensor_scalar_add(qt[:, 0, :], ps[:, 0:N],
                                            qkb_sb[:, mt:mt + 1])
                st[(i, "qt", mt)] = qt
            else:
                kt = ktp.tile([128, 2, N], fp8, tag="kt")
                nc.gpsimd.memset(kt[:, 1, :], 0.0)
                nc.scalar.copy(kt[:, 0, :], ps[:, 0:N])
                st[(i, "kt", mt - 6)] = kt

        def emit_vproj(i, tt, halfk):
            xb = st[(i, "xb")]
            if tt == 0 and halfk == 0:
                vpad = vpp.tile([128, 4, H, 65], bf16, tag="vpad")
                nc.vector.memset(vpad[:, :, :, 64:65], 1.0)
                st[(i, "vpad")] = vpad
                vpad4 = vp4.tile([128, 6, 65], bf16, tag="vpad4")
                nc.vector.memset(vpad4[:, :, 64:65], 1.0)
                st[(i, "vpad4")] = vpad4
            t0, tsz = JT[tt]
            if halfk == 0:
                st[(i, "vps", tt)] = sps.tile([128, 12, 64], f32, name="vps",
                                              tag="s")
            ps = st[(i, "vps", tt)]
            for kc in range(3 * halfk, 3 * halfk + 3):
                if tt < 4:
                    for g0, gn in [(0, 8), (8, 4)]:
                        nc.tensor.matmul(
                            ps[0:tsz, g0:g0 + gn, :],
                            lhsT=xb[:, kc, t0:t0 + tsz],
                            rhs=wv_sb[:, kc, g0 * 64:(g0 + gn) * 64],
                            start=(kc == 0), stop=(kc == 5))
                else:
                    # tail tokens: even heads -> psum partitions 0:64,
                    # odd heads -> 64:128 (paired layout for shared tail)
                    for par in range(2):
                        nc.tensor.matmul(
                            ps[64 * par:64 * par + 64, 0:6, :],
                            lhsT=xb[:, kc, t0:t0 + tsz],
                            rhs=wv_sb[:, kc, 384 * par:384 * par + 384],
                            start=(kc == 0), stop=(kc == 5),
                            skip_group_check=True)
            if halfk == 1:
                eng = nc.vector
                if tt < 4:
                    eng.tensor_copy(st[(i, "vpad")][0:tsz, tt, :, 0:64],
                                    ps[0:tsz, :, :])
                else:
                    eng.tensor_copy(st[(i, "vpad4")][:, :, 0:64],
                                    ps[:, 0:6, :])
                del st[(i, "vps", tt)]

        def emit_attn_qk(i, h, jt):
            base = (h % 2) * 64
            qv = st[(i, "qt", h // 2)]
            kv = st[(i, "kt", h // 2)]
            j0, jsz = JT[jt]
            s = sps.tile([128, 768], f32, name="satt", tag="s")
            for n0, nsz in [(0, 512), (512, 64)]:
                nc.tensor.matmul(
                    s[0:jsz, n0:n0 + nsz],
                    lhsT=kv[base:base + 64, :, j0:j0 + jsz],
                    rhs=qv[base:base + 64, :, n0:n0 + nsz].broadcast_to(
                        (64, 2, nsz)),
                    start=True, stop=True, perf_mode=DR)
            p2 = p2p.tile([128, N], bf16, tag="p2")
            nc.scalar.activation(p2[0:jsz, :], s[0:jsz, 0:N], EXP,
                                 scale=EXPSCALE)
            eng = nc.vector if jt % 2 == 0 else nc.gpsimd
            eng.tensor_mul(p2[0:jsz, :], p2[0:jsz, :],
                           expb_sb[0:jsz, h, jt, :])
            st[(i, "p2", h, jt)] = p2

        def emit_attn_qk_tail(i, hp):
            """jt=4 for both heads of pair hp: shared psum + one exp+mult."""
            he, ho = 2 * hp, 2 * hp + 1
            j0, jsz = JT[4]
            s = sps.tile([128, 768], f32, name="satt", tag="s")
            qv = st[(i, "qt", hp)]
            kv = st[(i, "kt", hp)]
            for n0, nsz in [(0, 512), (512, 64)]:
                # even head: DoubleRow into partitions 0:64
                nc.tensor.matmul(
                    s[0:jsz, n0:n0 + nsz],
                    lhsT=kv[0:64, :, j0:j0 + jsz],
                    rhs=qv[0:64, :, n0:n0 + nsz].broadcast_to((64, 2, nsz)),
                    start=True, stop=True, perf_mode=DR,
                    skip_group_check=True)
                # odd head: plain matmul into partitions 64:128 (DoubleRow
                # with dst partition base 64 fails the ISA check)
                nc.tensor.matmul(
                    s[64:64 + jsz, n0:n0 + nsz],
                    lhsT=kv[64:128, 0, j0:j0 + jsz],
                    rhs=qv[64:128, 0, n0:n0 + nsz],
                    start=True, stop=True,
                    skip_group_check=True)
            p2 = p2p.tile([128, N], bf16, tag="p2")
            nc.scalar.activation(p2[:, :], s[:, 0:N], EXP, scale=EXPSCALE)
            nc.vector.tensor_mul(p2[:, :], p2[:, :], expb4_sb[:, hp, :])
            st[(i, "p2", he, 4)] = p2
            st[(i, "p2", ho, 4)] = p2

        def emit_attn_pv(i, h, jt):
            j0, jsz = JT[jt]
            if jt == 0:
                st[(i, "od")] = bigp.tile([65, N], f32, name="od", tag="od")
            odp = st[(i, "od")]
            p2 = st.pop((i, "p2", h, jt))
            if jt < 4:
                lhsT = st[(i, "vpad")][0:jsz, jt, _vhp(h), :]
                rhs = [p2[0:jsz, n0:n0 + nsz] for n0, nsz in
                       [(0, 512), (512, 64)]]
            else:
                base = (h % 2) * 64
                lhsT = st[(i, "vpad4")][base:base + 64, h // 2, :]
                rhs = [p2[base:base + 64, n0:n0 + nsz] for n0, nsz in
                       [(0, 512), (512, 64)]]
            for (n0, nsz), r in zip([(0, 512), (512, 64)], rhs):
                nc.tensor.matmul(
                    odp[0:65, n0:n0 + nsz], lhsT=lhsT, rhs=r,
                    start=(jt == 0), stop=(jt == 4),
                    skip_group_check=True)
            if jt == 4:
                hh = h % 6
                if hh == 0:
                    st[(i, "odsall", h // 6)] = odsp.tile(
                        [65, 2, 3, N], bf16, name="odsall", tag="odsall")
                odsall = st[(i, "odsall", h // 6)]
                nc.vector.tensor_copy(odsall[:, hh % 2, hh // 2, :],
                                      odp[:, :])

        def emit_division(i, half):
            odsall = st[(i, "odsall", half)]
            if half == 0:
                ot = otp.tile([128, 6, N], bf16, tag="ot")
                st[(i, "ot")] = ot
            ot = st[(i, "ot")]
            d6 = small1.tile([6, N], bf16, tag="d6")
            nc.sync.dma_start(d6[:, :], odsall[64:65, :, :, :])
            rf32 = small1.tile([6, N], f32, tag="rf32")
            nc.vector.reciprocal(rf32[:, :], d6[:, :])
            rbf = small1.tile([6, N], bf16, tag="rbf")
            nc.vector.tensor_copy(rbf[:, :], rf32[:, :])
            rdram = dramp.tile([1, 6, N], bf16, tag="rdram")
            nc.sync.dma_start(rdram[0, :, :], rbf[:, :])
            rb6 = rbp.tile([64, 2, 3, N], bf16, tag="rb6")
            nc.sync.dma_start(rb6[:, :, :, :],
                              rdram[:, :, :].broadcast_to((64, 6, N)))
            for par in range(2):
                nc.vector.tensor_mul(odsall[0:64, par, :, :],
                                     odsall[0:64, par, :, :],
                                     rb6[:, par, :, :])
            nc.sync.dma_start(ot[0:64, 3 * half:3 * half + 3, :],
                              odsall[0:64, 0, :, :])
            nc.sync.dma_start(ot[64:128, 3 * half:3 * half + 3, :],
                              odsall[0:64, 1, :, :])

        def emit_proj(i, b, mt):
            ot = st[(i, "ot")]
            if mt == 0:
                st[(i, "y6")] = yp.tile([128, 6, N], bf16, name="y6",
                                        tag="y6")
            y6 = st[(i, "y6")]
            ps = sps.tile([128, 768], f32, name="pps", tag="s")
            for kc in range(6):
                for n0, nsz in [(0, 512), (512, 64)]:
                    nc.tensor.matmul(
                        ps[:, n0:n0 + nsz],
                        lhsT=wp_sb[:, kc, mt * 128:(mt + 1) * 128],
                        rhs=ot[:, kc, n0:n0 + nsz],
                        start=(kc == 0), stop=(kc == 5))
            nc.vector.tensor_scalar_add(y6[:, mt, :], ps[:, 0:N],
                                         pb2_sb[:, mt:mt + 1])
            if mt == 5:
                nc.sync.dma_start(out[:, :, b * N:(b + 1) * N], y6[:, :, :])

        # ---- head-pair software-pipelined emission ----
        from collections import deque
        batches = [(r * BL + b, b) for r in range(reps) for b in range(BL)]
        nbat = len(batches)
        pairs = [(i, b, hp) for (i, b) in batches for hp in range(6)]
        NP = len(pairs)

        # fill schedule: fills[(i, hp)] consumed during the slot whose
        # PV-phase is pair (i, hp)
        fills = {}
        for idx, (i, b) in enumerate(batches):
            items = []
            if idx + 1 < nbat:
                i2, b2 = batches[idx + 1]
                items.append(("s0", lambda i2=i2, b2=b2:
                              emit_load_xb(i2, b2)))
            if idx > 0:
                ip, bp = batches[idx - 1]
                for mt in range(6):
                    items.append(("x", lambda ip=ip, bp=bp, mt=mt:
                                  emit_proj(ip, bp, mt)))
            if idx + 1 < nbat:
                i2, b2 = batches[idx + 1]
                order = [("qk", 0), ("qk", 6), ("qk", 1), ("qk", 7),
                         ("v", 0), ("qk", 2), ("qk", 8), ("v", 1),
                         ("qk", 3), ("qk", 9), ("v", 2), ("qk", 4),
                         ("qk", 10), ("v", 3), ("qk", 5), ("qk", 11),
                         ("v", 4)]
                for kind, a in order:
                    if kind == "qk":
                        items.append(("x", lambda i2=i2, a=a:
                                      emit_qkproj(i2, a)))
                    else:
                        items.append(("x", lambda i2=i2, a=a:
                                      (emit_vproj(i2, a, 0),
                                       emit_vproj(i2, a, 1))))
            # distribute: slot 0 gets the xb load only; rest spread 1..5
            fl = {hp: [] for hp in range(6)}
            rest = []
            for kind, f in items:
                if kind == "s0":
                    fl[0].append(f)
                else:
                    rest.append(f)
            for n, f in enumerate(rest):
                fl[1 + (n * 5) // max(len(rest), 1)].append(f)
            for hp in range(6):
                fills[(i, hp)] = deque(fl[hp])

        load_wqk()
        load_wv_qkb()
        emit_load_xb(0, 0)
        for mt in range(12):
            emit_qkproj(0, mt)
        for tt in range(5):
            emit_vproj(0, tt, 0)
            emit_vproj(0, tt, 1)
        load_wp_expb()

        def qphase(s, k):
            """k-th QK-emission step (0..9) of the Q-side pair of slot s."""
            if s >= NP:
                return
            iq, bq, hq = pairs[s]
            he, ho = 2 * hq, 2 * hq + 1
            seq = [(he, 0), (ho, 0), (he, 1), (ho, 1), (he, 2),
                   (ho, 2), (he, 3), (ho, 3), (he, 4), (ho, 4)]
            h, jt = seq[k]
            if jt == 4:
                if h == ho:
                    emit_attn_qk_tail(iq, hq)
            else:
                emit_attn_qk(iq, h, jt)

        gfill = deque()

        for s in range(NP + 1):
            P = pairs[s - 1] if s >= 1 else None
            if P:
                gfill.extend(fills.get((P[0], P[2]), ()))
            f = gfill

            def pop():
                if f:
                    f.popleft()()

            pop()
            qphase(s, 0)
            if P:
                ip, bp, hp = P
                pe, po = 2 * hp, 2 * hp + 1
                emit_attn_pv(ip, pe, 0)
                qphase(s, 1)
                emit_attn_pv(ip, pe, 1)
                qphase(s, 2)
                emit_attn_pv(ip, pe, 2)
                pop()
                qphase(s, 3)
                emit_attn_pv(ip, pe, 3)
                qphase(s, 4)
                emit_attn_pv(ip, pe, 4)
                pop()
                qphase(s, 5)
                emit_attn_pv(ip, po, 0)
                qphase(s, 6)
                emit_attn_pv(ip, po, 1)
                qphase(s, 7)
                emit_attn_pv(ip, po, 2)
                pop()
                qphase(s, 8)
                emit_attn_pv(ip, po, 3)
                qphase(s, 9)
                emit_attn_pv(ip, po, 4)
                if hp == 5 and (s == NP or pairs[s][0] != ip):
                    while f:
                        f.popleft()()
                if hp == 2:
                    emit_division(ip, 0)
                elif hp == 5:
                    emit_division(ip, 1)
            else:
                for k in range(1, 10):
                    qphase(s, k)

        ilast, blast = batches[-1]
        for mt in range(6):
            emit_proj(ilast, blast, mt)

    nc.compile()
    _cache[key] = nc
    return nc


def _prep_inputs(x, qkv_w, q_bias, v_bias, rpb_table, rel_idx, proj_w, proj_b):
    x = np.asarray(x, np.float32)
    qkv_w = np.asarray(qkv_w, np.float32)
    q_bias = np.asarray(q_bias, np.float32)
    v_bias = np.asarray(v_bias, np.float32)
    rpb_table = np.asarray(rpb_table, np.float32)
    rel_idx = np.asarray(rel_idx)
    proj_w = np.asarray(proj_w, np.float32)
    proj_b = np.asarray(proj_b, np.float32)

    # q/k weights: natural magnitude (no attention scale), x WS for fp8,
    # packed [128, 3, 2, 1536]
    wqk_f = (qkv_w[:2 * C].T * WS).astype(F8)                       # [768,1536]
    wqk_np = np.ascontiguousarray(
        wqk_f.reshape(3, 2, 128, 2 * C).transpose(2, 0, 1, 3))
    qb = np.concatenate([q_bias * WS, np.zeros(C, np.float32)])     # [1536]
    qkb_np = np.ascontiguousarray(qb.reshape(12, 128).T)            # [128, 12]
    # wv columns parity-major: [even heads | odd heads]
    wv_f = qkv_w[2 * C:].T.astype(BF16)                             # [768, 768]
    wv_pm = np.ascontiguousarray(
        wv_f.reshape(C, 12, 64)[:, [0, 2, 4, 6, 8, 10, 1, 3, 5, 7, 9, 11],
                                :].reshape(C, C))
    wv_np = np.ascontiguousarray(
        wv_pm.reshape(6, 128, C).transpose(1, 0, 2))
    wp_np = np.ascontiguousarray(
        proj_w.T.astype(BF16).reshape(6, 128, C).transpose(1, 0, 2))
    pb_eff = (proj_b + proj_w @ v_bias).astype(np.float32)          # [768]
    pb2_np = np.ascontiguousarray(pb_eff.reshape(6, 128).T)         # [128, 6]
    et = np.exp(rpb_table)                     # [2209, 12]
    idx = np.clip(np.asarray(rel_idx, np.int64), 0, et.shape[0] - 1)
    g = et[idx]                                # [576i, 576j, 12]
    ghji = np.ascontiguousarray(g.transpose(2, 1, 0)).astype(BF16)  # [h, j, i]
    expb_np = np.zeros((128, H, 4, N), BF16)
    for jt, (j0, jsz) in enumerate(JT[:4]):
        expb_np[:, :, jt, :] = ghji[:, j0:j0 + jsz, :].transpose(1, 0, 2)
    # tail (j 512:576) packed per head pair: even head partitions 0:64
    expb4_np = np.zeros((128, 6, N), BF16)
    for hp in range(6):
        expb4_np[0:64, hp, :] = ghji[2 * hp, 512:576, :]
        expb4_np[64:128, hp, :] = ghji[2 * hp + 1, 512:576, :]

    in_maps = []
    for ci in range(NCORES):
        xc = x[ci * BL:(ci + 1) * BL]          # [8, 576, 768]
        xf = np.ascontiguousarray(
            xc.transpose(2, 0, 1).reshape(C, NTOK))
        xT_np = np.ascontiguousarray(
            xf.astype(BF16).reshape(6, 128, NTOK).transpose(1, 0, 2))
        xT8_np = np.ascontiguousarray(
            xf.astype(F8).reshape(3, 2, 128, NTOK).transpose(2, 0, 1, 3))
        in_maps.append({
            "xT": xT_np, "xT8": xT8_np,
            "wqk": wqk_np, "qkb": qkb_np,
            "wv": wv_np, "wp": wp_np, "pb2": pb2_np,
            "expb": expb_np, "expb4": expb4_np,
        })
    return in_maps


def kernel(x, qkv_w, q_bias, v_bias, rpb_table, rel_idx, proj_w, proj_b,
           _want_profile=False):
    in_maps = _prep_inputs(x, qkv_w, q_bias, v_bias, rpb_table, rel_idx,
                           proj_w, proj_b)
    nc = _build()
    from concourse.bass_utils import run_bass_kernel_spmd
    res = run_bass_kernel_spmd(nc, in_maps, core_ids=list(range(NCORES)),
                               trace=False)
    outs = [np.asarray(r["out"], np.float32).transpose(1, 0, 2).reshape(
        C, NTOK).T.reshape(BL, N, C) for r in res.results]
    y = np.concatenate(outs, 0)
    if _want_profile:
        return y, res
    return y


# revision 29
# speedup vs baseline: 5.5394x; 1.3243x over previous
"""Trainium2 Bass kernel: Swin-style window attention with relative position bias.

Self-contained: hardcodes B=64, N=576, C=768, H=12. Shards batch over 8 cores.

Per-core design (fully on-device compute; host only reshapes/sharding):
 - q/k projection in fp8e4 with DoubleRow perf mode (2 k-tiles of 128 per
   matmul at 0.5 cycles/row); weights pre-scaled by 16 (host) to dodge fp8
   subnormals; q/k sbuf tiles fp8 at 16x natural scale.
 - S_T[j,i] = k^T q per (batch, head) fp8 DoubleRow: k tile [d(64), 2, j]
   with slot1 zeroed, q broadcast stride-0 on the slot dim; the 16*16 and
   1/sqrt(d) factors fold into exp's scale = 1/2048.
 - softmax without max-subtract (logits bounded): P = exp(S_T/2048) * expb
   (exp written into the p2 tile, multiplied in place). The j-tail (j 512:576)
   of a head PAIR shares one psum tile (even head on partitions 0:64, odd on
   64:128) so its exp+mult are one instruction per pair, not two.
 - PV bf16: oD[65, i] = [v | 1]^T P_T accumulated over j-chunks; row 64 is
   the softmax denominator. v stored parity-major; the v-tail of a pair is
   packed [128, pair, 65] to match the shared tail layout.
 - Per-half division: gather 6 denom rows via one DMA, reciprocal, dram
   round-trip broadcast to 64 partitions (one stride-0 DMA), multiply in
   place, assemble oT via 2 partition-shift DMAs; project with bias.
 - All dram tensors host-packed to sbuf tile layout => one DMA per load.
 - Emission is software-pipelined per head pair: PV-phase of pair n runs
   interleaved with QK-phase of pair n+1; projection / next-batch qkv
   fills are scheduled into known PE stall points.
"""
import sys

sys.path.insert(0, "/opt/trn_rl_repo")

import numpy as np
import ml_dtypes

BF16 = ml_dtypes.bfloat16
F8 = ml_dtypes.float8_e4m3

B, N, C = 64, 576, 768
H, D = 12, 64
NCORES = 8
BL = B // NCORES           # 8 batches per core
NTOK = BL * N              # 4608 tokens per core
WS = 16.0                  # fp8 weight pre-scale (host)
EXPSCALE = 1.0 / (WS * WS * (D ** 0.5))  # = 1/2048

# token/j tiles of N=576: 4x128 + 1x64
JT = [(0, 128), (128, 128), (256, 128), (384, 128), (512, 64)]

_cache = {}


def _vhp(h):
    """Head index inside vpad/vps free dim (parity-major)."""
    return (h % 2) * 6 + h // 2


def _build(reps=1):
    key = ("nc", reps)
    if key in _cache:
        return _cache[key]
    from contextlib import ExitStack
    import concourse.tile as tile
    from concourse import bacc, mybir

    f32 = mybir.dt.float32
    bf16 = mybir.dt.bfloat16
    fp8 = mybir.dt.float8e4
    DR = mybir.MatmulPerfMode.DoubleRow
    EXP = mybir.ActivationFunctionType.Exp

    nc = bacc.Bacc("TRN2", target_bir_lowering=False, debug=False,
                   num_devices=NCORES)
    xT = nc.dram_tensor("xT", [128, 6, NTOK], bf16, kind="ExternalInput").ap()
    xT8 = nc.dram_tensor("xT8", [128, 3, 2, NTOK], fp8,
                         kind="ExternalInput").ap()
    wqk = nc.dram_tensor("wqk", [128, 3, 2, 2 * C], fp8,
                         kind="ExternalInput").ap()
    qkb = nc.dram_tensor("qkb", [128, 12], f32, kind="ExternalInput").ap()
    wv = nc.dram_tensor("wv", [128, 6, C], bf16, kind="ExternalInput").ap()
    wp = nc.dram_tensor("wp", [128, 6, C], bf16, kind="ExternalInput").ap()
    pb2 = nc.dram_tensor("pb2", [128, 6], f32, kind="ExternalInput").ap()
    expb = nc.dram_tensor("expb", [128, H, 4, N], bf16,
                          kind="ExternalInput").ap()
    expb4 = nc.dram_tensor("expb4", [128, 6, N], bf16,
                           kind="ExternalInput").ap()
    out = nc.dram_tensor("out", [128, 6, NTOK], bf16,
                         kind="ExternalOutput").ap()

    with tile.TileContext(nc) as tc, ExitStack() as ctx:
        ent = ctx.enter_context
        const = ent(tc.tile_pool(name="const", bufs=1))
        sps = ent(tc.tile_pool(name="sps", bufs=3, space="PSUM"))
        bigp = ent(tc.tile_pool(name="bigp", bufs=1, space="PSUM"))
        xbp = ent(tc.tile_pool(name="xb", bufs=2))
        xb8p = ent(tc.tile_pool(name="xb8", bufs=2))
        qtp = ent(tc.tile_pool(name="qt", bufs=10))
        ktp = ent(tc.tile_pool(name="kt", bufs=10))
        vpp = ent(tc.tile_pool(name="vpad", bufs=2))
        vp4 = ent(tc.tile_pool(name="vpad4", bufs=2))
        p2p = ent(tc.tile_pool(name="p2p", bufs=12))
        odsp = ent(tc.tile_pool(name="odsall", bufs=2))
        otp = ent(tc.tile_pool(name="ot", bufs=2))
        yp = ent(tc.tile_pool(name="y", bufs=2))
        small1 = ent(tc.tile_pool(name="small1", bufs=1))
        rbp = ent(tc.tile_pool(name="rb", bufs=1))
        dramp = ent(tc.tile_pool(name="dram", bufs=2, space="DRAM"))

        # ---- constant tiles (single packed DMAs, emitted lazily below) ----
        wqk_sb = const.tile([128, 3, 2, 2 * C], fp8)
        wv_sb = const.tile([128, 6, C], bf16)
        wp_sb = const.tile([128, 6, C], bf16)
        qkb_sb = const.tile([128, 12], f32)
        pb2_sb = const.tile([128, 6], f32)
        expb_sb = const.tile([128, H, 4, N], bf16)
        expb4_sb = const.tile([128, 6, N], bf16)

        def load_wqk():
            nc.gpsimd.dma_start(wqk_sb[:, :, :, :], wqk[:, :, :, :])

        def load_wv_qkb():
            nc.gpsimd.dma_start(wv_sb[:, :, :], wv[:, :, :])
            nc.gpsimd.dma_start(qkb_sb[:, :], qkb[:, :])
            nc.gpsimd.dma_start(pb2_sb[:, :], pb2[:, :])

        def load_wp_expb():
            nc.gpsimd.dma_start(wp_sb[:, :, :], wp[:, :, :])
            nc.gpsimd.dma_start(expb_sb[:, :, :, :], expb[:, :, :, :])
            nc.gpsimd.dma_start(expb4_sb[:, :, :], expb4[:, :, :])

        # ---- per-batch state (keyed by flattened batch index) ----
        st = {}

        def emit_load_xb(i, b):
            xb = xbp.tile([128, 6, N], bf16, tag="xb")
            nc.sync.dma_start(xb[:, :, :], xT[:, :, b * N:(b + 1) * N])
            xb8 = xb8p.tile([128, 3, 2, N], fp8, tag="xb8")
            nc.gpsimd.dma_start(xb8[:, :, :, :],
                                xT8[:, :, :, b * N:(b + 1) * N])
            st[(i, "xb")] = xb
            st[(i, "xb8")] = xb8

        def emit_qkproj(i, mt):
            xb8 = st[(i, "xb8")]
            ps = sps.tile([128, 768], f32, name="qps", tag="s")
            for kc2 in range(3):
                for n0, nsz in [(0, 512), (512, 64)]:
                    nc.tensor.matmul(
                        ps[:, n0:n0 + nsz],
                        lhsT=wqk_sb[:, kc2, :, mt * 128:(mt + 1) * 128],
                        rhs=xb8[:, kc2, :, n0:n0 + nsz],
                        start=(kc2 == 0), stop=(kc2 == 2),
                        perf_mode=DR)
            if mt < 6:
                qt = qtp.tile([128, 1, N], fp8, tag="qt")
                nc.vector.tensor_scalar_add(qt[:, 0, :], ps[:, 0:N],
                                            qkb_sb[:, mt:mt + 1])
                st[(i, "qt", mt)] = qt
            else:
                kt = ktp.tile([128, 2, N], fp8, tag="kt")
                nc.gpsimd.memset(kt[:, 1, :], 0.0)
                nc.scalar.copy(kt[:, 0, :], ps[:, 0:N])
                st[(i, "kt", mt - 6)] = kt

        def emit_vproj(i, tt, halfk):
            xb = st[(i, "xb")]
            if tt == 0 and halfk == 0:
                vpad = vpp.tile([128, 4, H, 65], bf16, tag="vpad")
                nc.vector.memset(vpad[:, :, :, 64:65], 1.0)
                st[(i, "vpad")] = vpad
                vpad4 = vp4.tile([128, 6, 65], bf16, tag="vpad4")
                nc.vector.memset(vpad4[:, :, 64:65], 1.0)
                st[(i, "vpad4")] = vpad4
            t0, tsz = JT[tt]
            if halfk == 0:
                st[(i, "vps", tt)] = sps.tile([128, 12, 64], f32, name="vps",
                                              tag="s")
            ps = st[(i, "vps", tt)]
            for kc in range(3 * halfk, 3 * halfk + 3):
                if tt < 4:
                    for g0, gn in [(0, 8), (8, 4)]:
                        nc.tensor.matmul(
                            ps[0:tsz, g0:g0 + gn, :],
                            lhsT=xb[:, kc, t0:t0 + tsz],
                            rhs=wv_sb[:, kc, g0 * 64:(g0 + gn) * 64],
                            start=(kc == 0), stop=(kc == 5))
                else:
                    # tail tokens: even heads -> psum partitions 0:64,
                    # odd heads -> 64:128 (paired layout for shared tail)
                    for par in range(2):
                        nc.tensor.matmul(
                            ps[64 * par:64 * par + 64, 0:6, :],
                            lhsT=xb[:, kc, t0:t0 + tsz],
                            rhs=wv_sb[:, kc, 384 * par:384 * par + 384],
                            start=(kc == 0), stop=(kc == 5),
                            skip_group_check=True)
            if halfk == 1:
                eng = nc.vector
                if tt < 4:
                    eng.tensor_copy(st[(i, "vpad")][0:tsz, tt, :, 0:64],
                                    ps[0:tsz, :, :])
                else:
                    eng.tensor_copy(st[(i, "vpad4")][:, :, 0:64],
                                    ps[:, 0:6, :])
                del st[(i, "vps", tt)]

        def emit_attn_qk(i, h, jt):
            base = (h % 2) * 64
            qv = st[(i, "qt", h // 2)]
            kv = st[(i, "kt", h // 2)]
            j0, jsz = JT[jt]
            s = sps.tile([128, 768], f32, name="satt", tag="s")
            for n0, nsz in [(0, 512), (512, 64)]:
                nc.tensor.matmul(
                    s[0:jsz, n0:n0 + nsz],
                    lhsT=kv[base:base + 64, :, j0:j0 + jsz],
                    rhs=qv[base:base + 64, :, n0:n0 + nsz].broadcast_to(
                        (64, 2, nsz)),
                    start=True, stop=True, perf_mode=DR)
            p2 = p2p.tile([128, N], bf16, tag="p2")
            nc.scalar.activation(p2[0:jsz, :], s[0:jsz, 0:N], EXP,
                                 scale=EXPSCALE)
            eng = nc.vector if jt % 2 == 0 else nc.gpsimd
            eng.tensor_mul(p2[0:jsz, :], p2[0:jsz, :],
                           expb_sb[0:jsz, h, jt, :])
            st[(i, "p2", h, jt)] = p2

        def emit_attn_qk_tail(i, hp):
            """jt=4 for both heads of pair hp: shared psum + one exp+mult."""
            he, ho = 2 * hp, 2 * hp + 1
            j0, jsz = JT[4]
            s = sps.tile([128, 768], f32, name="satt", tag="s")
            qv = st[(i, "qt", hp)]
            kv = st[(i, "kt", hp)]
            for n0, nsz in [(0, 512), (512, 64)]:
                # even head: DoubleRow into partitions 0:64
                nc.tensor.matmul(
                    s[0:jsz, n0:n0 + nsz],
                    lhsT=kv[0:64, :, j0:j0 + jsz],
                    rhs=qv[0:64, :, n0:n0 + nsz].broadcast_to((64, 2, nsz)),
                    start=True, stop=True, perf_mode=DR,
                    skip_group_check=True)
                # odd head: plain matmul into partitions 64:128 (DoubleRow
                # with dst partition base 64 fails the ISA check)
                nc.tensor.matmul(
                    s[64:64 + jsz, n0:n0 + nsz],
                    lhsT=kv[64:128, 0, j0:j0 + jsz],
                    rhs=qv[64:128, 0, n0:n0 + nsz],
                    start=True, stop=True,
                    skip_group_check=True)
            p2 = p2p.tile([128, N], bf16, tag="p2")
            nc.scalar.activation(p2[:, :], s[:, 0:N], EXP, scale=EXPSCALE)
            nc.vector.tensor_mul(p2[:, :], p2[:, :], expb4_sb[:, hp, :])
            st[(i, "p2", he, 4)] = p2
            st[(i, "p2", ho, 4)] = p2

        def emit_attn_pv(i, h, jt):
            j0, jsz = JT[jt]
            if jt == 0:
                st[(i, "od")] = bigp.tile([65, N], f32, name="od", tag="od")
            odp = st[(i, "od")]
            p2 = st.pop((i, "p2", h, jt))
            if jt < 4:
                lhsT = st[(i, "vpad")][0:jsz, jt, _vhp(h), :]
                rhs = [p2[0:jsz, n0:n0 + nsz] for n0, nsz in
                       [(0, 512), (512, 64)]]
            else:
                base = (h % 2) * 64
                lhsT = st[(i, "vpad4")][base:base + 64, h // 2, :]
                rhs = [p2[base:base + 64, n0:n0 + nsz] for n0, nsz in
                       [(0, 512), (512, 64)]]
            for (n0, nsz), r in zip([(0, 512), (512, 64)], rhs):
                nc.tensor.matmul(
                    odp[0:65, n0:n0 + nsz], lhsT=lhsT, rhs=r,
                    start=(jt == 0), stop=(jt == 4),
                    skip_group_check=True)
            if jt == 4:
                hh = h % 6
                if hh == 0:
                    st[(i, "odsall", h // 6)] = odsp.tile(
                        [65, 2, 3, N], bf16, name="odsall", tag="odsall")
                odsall = st[(i, "odsall", h // 6)]
                nc.vector.tensor_copy(odsall[:, hh % 2, hh // 2, :],
                                      odp[:, :])

        def emit_division(i, half):
            odsall = st[(i, "odsall", half)]
            if half == 0:
                ot = otp.tile([128, 6, N], bf16, tag="ot")
                st[(i, "ot")] = ot
            ot = st[(i, "ot")]
            d6 = small1.tile([6, N], bf16, tag="d6")
            nc.sync.dma_start(d6[:, :], odsall[64:65, :, :, :])
            rf32 = small1.tile([6, N], f32, tag="rf32")
            nc.vector.reciprocal(rf32[:, :], d6[:, :])
            rbf = small1.tile([6, N], bf16, tag="rbf")
            nc.vector.tensor_copy(rbf[:, :], rf32[:, :])
            rdram = dramp.tile([1, 6, N], bf16, tag="rdram")
            nc.sync.dma_start(rdram[0, :, :], rbf[:, :])
            rb6 = rbp.tile([64, 2, 3, N], bf16, tag="rb6")
            nc.sync.dma_start(rb6[:, :, :, :],
                              rdram[:, :, :].broadcast_to((64, 6, N)))
            for par in range(2):
                nc.vector.tensor_mul(odsall[0:64, par, :, :],
                                     odsall[0:64, par, :, :],
                                     rb6[:, par, :, :])
            nc.sync.dma_start(ot[0:64, 3 * half:3 * half + 3, :],
                              odsall[0:64, 0, :, :])
            nc.sync.dma_start(ot[64:128, 3 * half:3 * half + 3, :],
                              odsall[0:64, 1, :, :])

        def emit_proj(i, b, mt):
            ot = st[(i, "ot")]
            if mt == 0:
                st[(i, "y6")] = yp.tile([128, 6, N], bf16, name="y6",
                                        tag="y6")
            y6 = st[(i, "y6")]
            ps = sps.tile([128, 768], f32, name="pps", tag="s")
            for kc in range(6):
                for n0, nsz in [(0, 512), (512, 64)]:
                    nc.tensor.matmul(
                        ps[:, n0:n0 + nsz],
                        lhsT=wp_sb[:, kc, mt * 128:(mt + 1) * 128],
                        rhs=ot[:, kc, n0:n0 + nsz],
                        start=(kc == 0), stop=(kc == 5))
            nc.vector.tensor_scalar_add(y6[:, mt, :], ps[:, 0:N],
                                         pb2_sb[:, mt:mt + 1])
            if mt == 5:
                nc.sync.dma_start(out[:, :, b * N:(b + 1) * N], y6[:, :, :])

        # ---- head-pair software-pipelined emission ----
        from collections import deque
        batches = [(r * BL + b, b) for r in range(reps) for b in range(BL)]
        nbat = len(batches)
        pairs = [(i, b, hp) for (i, b) in batches for hp in range(6)]
        NP = len(pairs)

        # fill schedule: fills[(i, hp)] consumed during the slot whose
        # PV-phase is pair (i, hp)
        fills = {}
        for idx, (i, b) in enumerate(batches):
            items = []
            if idx + 1 < nbat:
                i2, b2 = batches[idx + 1]
                items.append(("s0", lambda i2=i2, b2=b2:
                              emit_load_xb(i2, b2)))
            if idx > 0:
                ip, bp = batches[idx - 1]
                for mt in range(6):
                    items.append(("x", lambda ip=ip, bp=bp, mt=mt:
                                  emit_proj(ip, bp, mt)))
            if idx + 1 < nbat:
                i2, b2 = batches[idx + 1]
                order = [("qk", 0), ("qk", 6), ("qk", 1), ("qk", 7),
                         ("v", 0), ("qk", 2), ("qk", 8), ("v", 1),
                         ("qk", 3), ("qk", 9), ("v", 2), ("qk", 4),
                         ("qk", 10), ("v", 3), ("qk", 5), ("qk", 11),
                         ("v", 4)]
                for kind, a in order:
                    if kind == "qk":
                        items.append(("x", lambda i2=i2, a=a:
                                      emit_qkproj(i2, a)))
                    else:
                        items.append(("x", lambda i2=i2, a=a:
                                      (emit_vproj(i2, a, 0),
                                       emit_vproj(i2, a, 1))))
            # distribute: slot 0 gets the xb load only; rest spread 1..5
            fl = {hp: [] for hp in range(6)}
            rest = []
            for kind, f in items:
                if kind == "s0":
                    fl[0].append(f)
                else:
                    rest.append(f)
            for n, f in enumerate(rest):
                fl[1 + (n * 5) // max(len(rest), 1)].append(f)
            for hp in range(6):
                fills[(i, hp)] = deque(fl[hp])

        load_wqk()
        load_wv_qkb()
        emit_load_xb(0, 0)
        for mt in range(12):
            emit_qkproj(0, mt)
        for tt in range(5):
            emit_vproj(0, tt, 0)
            emit_vproj(0, tt, 1)
        load_wp_expb()

        def qphase(s, k):
            """k-th QK-emission step (0..9) of the Q-side pair of slot s."""
            if s >= NP:
                return
            iq, bq, hq = pairs[s]
            he, ho = 2 * hq, 2 * hq + 1
            seq = [(he, 0), (ho, 0), (he, 1), (ho, 1), (he, 2),
                   (ho, 2), (he, 3), (ho, 3), (he, 4), (ho, 4)]
            h, jt = seq[k]
            if jt == 4:
                if h == ho:
                    emit_attn_qk_tail(iq, hq)
            else:
                emit_attn_qk(iq, h, jt)

        gfill = deque()

        for s in range(NP + 1):
            P = pairs[s - 1] if s >= 1 else None
            if P:
                gfill.extend(fills.get((P[0], P[2]), ()))
            f = gfill

            def pop():
                if f:
                    f.popleft()()

            pop()
            qphase(s, 0)
            if P:
                ip, bp, hp = P
                pe, po = 2 * hp, 2 * hp + 1
                emit_attn_pv(ip, pe, 0)
                qphase(s, 1)
                emit_attn_pv(ip, pe, 1)
                qphase(s, 2)
                emit_attn_pv(ip, pe, 2)
                pop()
                qphase(s, 3)
                emit_attn_pv(ip, pe, 3)
                qphase(s, 4)
                emit_attn_pv(ip, pe, 4)
                pop()
                qphase(s, 5)
                emit_attn_pv(ip, po, 0)
                qphase(s, 6)
                emit_attn_pv(ip, po, 1)
                qphase(s, 7)
                emit_attn_pv(ip, po, 2)
                pop()
                qphase(s, 8)
                emit_attn_pv(ip, po, 3)
                qphase(s, 9)
                emit_attn_pv(ip, po, 4)
                if hp == 5 and (s == NP or pairs[s][0] != ip):
                    while f:
                        f.popleft()()
                if hp == 2:
                    emit_division(ip, 0)
                elif hp == 5:
                    emit_division(ip, 1)
            else:
                for k in range(1, 10):
                    qphase(s, k)

        ilast, blast = batches[-1]
        for mt in range(6):
            emit_proj(ilast, blast, mt)

    nc.compile()
    _cache[key] = nc
    return nc


def _prep_inputs(x, qkv_w, q_bias, v_bias, rpb_table, rel_idx, proj_w, proj_b):
    x = np.asarray(x, np.float32)
    qkv_w = np.asarray(qkv_w, np.float32)
    q_bias = np.asarray(q_bias, np.float32)
    v_bias = np.asarray(v_bias, np.float32)
    rpb_table = np.asarray(rpb_table, np.float32)
    rel_idx = np.asarray(rel_idx)
    proj_w = np.asarray(proj_w, np.float32)
    proj_b = np.asarray(proj_b, np.float32)

    # q/k weights: natural magnitude (no attention scale), x WS for fp8,
    # packed [128, 3, 2, 1536]
    wqk_f = (qkv_w[:2 * C].T * WS).astype(F8)                       # [768,1536]
    wqk_np = np.ascontiguousarray(
        wqk_f.reshape(3, 2, 128, 2 * C).transpose(2, 0, 1, 3))
    qb = np.concatenate([q_bias * WS, np.zeros(C, np.float32)])     # [1536]
    qkb_np = np.ascontiguousarray(qb.reshape(12, 128).T)            # [128, 12]
    # wv columns parity-major: [even heads | odd heads]
    wv_f = qkv_w[2 * C:].T.astype(BF16)                             # [768, 768]
    wv_pm = np.ascontiguousarray(
        wv_f.reshape(C, 12, 64)[:, [0, 2, 4, 6, 8, 10, 1, 3, 5, 7, 9, 11],
                                :].reshape(C, C))
    wv_np = np.ascontiguousarray(
        wv_pm.reshape(6, 128, C).transpose(1, 0, 2))
    wp_np = np.ascontiguousarray(
        proj_w.T.astype(BF16).reshape(6, 128, C).transpose(1, 0, 2))
    pb_eff = (proj_b + proj_w @ v_bias).astype(np.float32)          # [768]
    pb2_np = np.ascontiguousarray(pb_eff.reshape(6, 128).T)         # [128, 6]
    et = np.exp(rpb_table)                     # [2209, 12]
    idx = np.clip(np.asarray(rel_idx, np.int64), 0, et.shape[0] - 1)
    g = et[idx]                                # [576i, 576j, 12]
    ghji = np.ascontiguousarray(g.transpose(2, 1, 0)).astype(BF16)  # [h, j, i]
    expb_np = np.zeros((128, H, 4, N), BF16)
    for jt, (j0, jsz) in enumerate(JT[:4]):
        expb_np[:, :, jt, :] = ghji[:, j0:j0 + jsz, :].transpose(1, 0, 2)
    # tail (j 512:576) packed per head pair: even head partitions 0:64
    expb4_np = np.zeros((128, 6, N), BF16)
    for hp in range(6):
        expb4_np[0:64, hp, :] = ghji[2 * hp, 512:576, :]
        expb4_np[64:128, hp, :] = ghji[2 * hp + 1, 512:576, :]

    in_maps = []
    for ci in range(NCORES):
        xc = x[ci * BL:(ci + 1) * BL]          # [8, 576, 768]
        xf = np.ascontiguousarray(
            xc.transpose(2, 0, 1).reshape(C, NTOK))
        xT_np = np.ascontiguousarray(
            xf.astype(BF16).reshape(6, 128, NTOK).transpose(1, 0, 2))
        xT8_np = np.ascontiguousarray(
            xf.astype(F8).reshape(3, 2, 128, NTOK).transpose(2, 0, 1, 3))
        in_maps.append({
            "xT": xT_np, "xT8": xT8_np,
            "wqk": wqk_np, "qkb": qkb_np,
            "wv": wv_np, "wp": wp_np, "pb2": pb2_np,
            "expb": expb_np, "expb4": expb4_np,
        })
    return in_maps


def kernel(x, qkv_w, q_bias, v_bias, rpb_table, rel_idx, proj_w, proj_b,
           _want_profile=False):
    in_maps = _prep_inputs(x, qkv_w, q_bias, v_bias, rpb_table, rel_idx,
                           proj_w, proj_b)
    nc = _build()
    from concourse.bass_utils import run_bass_kernel_spmd
    res = run_bass_kernel_spmd(nc, in_maps, core_ids=list(range(NCORES)),
                               trace=False)
    outs = [np.asarray(r["out"], np.float32).transpose(1, 0, 2).reshape(
        C, NTOK).T.reshape(BL, N, C) for r in res.results]
    y = np.concatenate(outs, 0)
    if _want_profile:
        return y, res
    return y
